# revision 19
# baseline (speedup 1.0000x reference)
"""BlockNTP transformer forward + cross-entropy loss on 8 trn2 NeuronCores.

v3: LayerNorm folded into matmul consumers so the PE never stalls on the LN
chain.  For row q with mean mu[q], rstd[q]:
  W^T h = (W^T x - colsum(W) * mu[q]) * rstd[q]
The colsum term is ONE extra rank-1 accumulation matmul per psum tile
(lhsT = -colsum chunk [1,128], rhs = mu16 [1,rows]); the rstd scale folds
into the existing psum-consumer DVE ops (Q mask-mult, K mult, V per-partition
tensor_scalar, w1 pre-gelu mult).  The PE starts qkv/w1 matmul groups right
after the residual adds; only the cheap DVE consumers wait on the rstd chain.

Sharding: cores 0-3 own batch elem 0, cores 4-7 elem 1; core (e, j) owns
256 rows (token rows [128j,128j+128) + mask rows 512+[128j,128j+128)).
Weights replicated (streamed bf16).  Per layer ONE fp8e4m3 K/V AllGather
(two 4-rank groups).  Layer-0 K/V precomputed host-side.  Masks degenerate
to all-or-nothing rows -> zeroed Q rows give exact uniform softmax.
Attention runs head PAIRS on disjoint PE row-groups; softmax denominators
ride in V's ones-row; normalization uses reciprocal_approx_fast via an SBUF
staging copy (custom DVE ops misread PSUM).  Q/K/V/E all fp8e4m3.
Target logits are computed locally per core; only sum-exp needs the final
8-rank x AllGather.
"""
import numpy as np
import ml_dtypes

import concourse.bass as bass
import concourse.mybir as mybir
import concourse.tile as tile
from concourse import bacc
from concourse.bass_utils import run_bass_kernel_spmd

B, T = 2, 512
D, H, DFF = 1024, 16, 4096
V, CSL = 32000, 16
NL, NDL = 4, 2
NLAYERS = NL + NDL
DH = D // H
S = 2 * T                    # 1024 rows per batch elem
NC = 8                       # cores
NG = 4                       # cores per elem
RPC = 256                    # rows per core (128 token + 128 mask)
VS = V // NC                 # 4000 vocab per core
F32 = mybir.dt.float32
BF16 = mybir.dt.bfloat16
FP8 = mybir.dt.float8e4
BF = ml_dtypes.bfloat16
F8 = ml_dtypes.float8_e4m3

KCH = 8 * RPC                # 2048: K^T part free elems per chunk
VCH = 2 * H * (DH + 1)       # 2080: V part free elems per chunk
KVFREE = KCH + VCH           # 4128

_CACHE = {}


def _build_nc(n_layers=NLAYERS):
    nc = bacc.Bacc("TRN2", target_bir_lowering=False, debug=False, num_devices=NC)

    x0 = nc.dram_tensor("x0", [D, RPC], F32, kind="ExternalInput")
    kv0 = nc.dram_tensor("kv0", [NG, 128, KVFREE], FP8, kind="ExternalInput")
    wqkvc = nc.dram_tensor("wqkvc", [NLAYERS, 6, 128, 8, 512], BF16,
                           kind="ExternalInput")
    woc = nc.dram_tensor("woc", [NLAYERS, 2, 128, 8, 512], BF16,
                         kind="ExternalInput")
    w1c = nc.dram_tensor("w1c", [NLAYERS, 8, 128, 8, 512], BF16,
                         kind="ExternalInput")
    w2c = nc.dram_tensor("w2c", [NLAYERS, 8, 128, 32, 128], BF16,
                         kind="ExternalInput")
    csqkv = nc.dram_tensor("csqkv", [NLAYERS, 1, 3 * D], BF16,
                           kind="ExternalInput")
    csw1 = nc.dram_tensor("csw1", [NLAYERS, 1, DFF], BF16, kind="ExternalInput")
    qm = nc.dram_tensor("qm", [2, RPC], F32, kind="ExternalInput")
    embc = nc.dram_tensor("embc", [8, 128, 8, 500], BF16, kind="ExternalInput")
    etT = nc.dram_tensor("etT", [128, 8, 128], BF16, kind="ExternalInput")
    sumexp_o = nc.dram_tensor("sumexp", [128, 8], F32, kind="ExternalOutput")
    tlogit_o = nc.dram_tensor("tlogit", [1, 128], F32, kind="ExternalOutput")

    with tile.TileContext(nc) as tc:
        with (
            tc.tile_pool(name="persist", bufs=1) as pp,
            tc.tile_pool(name="wpool", bufs=4) as wp,
            tc.tile_pool(name="epool", bufs=3) as ep,
            tc.tile_pool(name="tmp", bufs=2) as tp,
            tc.tile_pool(name="small", bufs=2) as sp,
            tc.tile_pool(name="psS", bufs=2, space="PSUM") as psS,
            tc.tile_pool(name="psA", bufs=2, space="PSUM") as psA,
            tc.tile_pool(name="psO", bufs=2, space="PSUM") as psO,
            tc.tile_pool(name="dram", bufs=2, space="DRAM") as dp,
        ):
            xT = pp.tile([128, 8, RPC], F32, name="xT")
            xb = pp.tile([128, 8, RPC], BF16, name="xb")
            QT = pp.tile([128, 8, RPC], FP8, name="QT")
            Kfull = pp.tile([128, 8, S], FP8, name="Kfull")
            Vfull = pp.tile([128, 8, H, DH + 1], FP8, name="Vfull")
            OT = pp.tile([128, 8, RPC], BF16, name="OT")
            G = pp.tile([128, 32, RPC], BF16, name="G")
            KTst = pp.tile([128, 8, RPC], FP8, name="KTst")
            Vst = pp.tile([128, 2, H, DH + 1], FP8, name="Vst")
            ones = pp.tile([128, 1], F32, name="ones")
            nc.vector.memset(ones[:], 1.0)
            ones16 = pp.tile([128, 1], BF16, name="ones16")
            nc.vector.memset(ones16[:], 1.0)
            ones_r = pp.tile([1, 128], F32, name="ones_r")
            nc.vector.memset(ones_r[:], 1.0)
            one11 = pp.tile([1, 1], F32, name="one11")
            nc.vector.memset(one11[:], 1.0)
            eps = pp.tile([1, 1], F32, name="eps")
            nc.vector.memset(eps[:], 1e-5)
            nc.vector.memset(Vst[:, :, :, DH : DH + 1], 1.0)
            masks = pp.tile([1, 2 * RPC], F32, name="masks")
            nc.sync.dma_start(masks[:, 0:RPC], qm.ap()[0:1, :])
            nc.sync.dma_start(masks[:, RPC : 2 * RPC], qm.ap()[1:2, :])
            masksB = pp.tile([128, 2, RPC], F32, name="masksB")
            for mi in range(2):
                mb = psA.tile([128, RPC], F32, name=f"mb{mi}", tag="A")
                nc.tensor.matmul(mb[:], ones_r[:],
                                 masks[:, mi * RPC : (mi + 1) * RPC],
                                 start=True, stop=True)
                nc.vector.tensor_copy(masksB[:, mi, :], mb[:])

            nc.sync.dma_start(xT[:], x0.ap().rearrange("(a p) c -> p a c", p=128))
            for a in range(8):
                nc.vector.tensor_copy(xb[:, a, :], xT[:, a, :])

            def ln_stats(li, which, mi=None, c0=0):
                """LN stats over D (partitions) of xb cols [c0,RPC)."""
                w = RPC - c0
                ps1 = psA.tile([1, RPC], F32, name=f"s1_{li}{which}", tag="A")
                ps2 = psA.tile([1, RPC], F32, name=f"s2_{li}{which}", tag="A")
                sqf = tp.tile([128, 8, RPC], F32, tag="lnsq", name=f"sq{li}{which}")
                for a in range(8):
                    xs = xb[:, a, c0:RPC]
                    nc.vector.tensor_tensor(sqf[:, a, 0:w], xs, xs,
                                            mybir.AluOpType.mult)
                for a in range(8):
                    nc.tensor.matmul(ps1[:, 0:w], ones16[:], xb[:, a, c0:RPC],
                                     start=(a == 0), stop=(a == 7))
                for a in range(8):
                    nc.tensor.matmul(ps2[:, 0:w], ones[:], sqf[:, a, 0:w],
                                     start=(a == 0), stop=(a == 7))
                mu = sp.tile([1, RPC], F32, tag="lnmu", name=f"mu{li}{which}")
                var = sp.tile([1, RPC], F32, tag="lnvar", name=f"var{li}{which}")
                nc.vector.tensor_scalar_mul(mu[:, 0:w], ps1[:, 0:w], 1.0 / D)
                nc.vector.tensor_scalar_mul(var[:, 0:w], ps2[:, 0:w], 1.0 / D)
                mu16 = sp.tile([1, RPC], BF16, tag="lnmu16", name=f"m16{li}{which}")
                nc.vector.tensor_copy(mu16[:, c0:RPC], mu[:, 0:w])
                msq = sp.tile([1, RPC], F32, tag="lnmsq", name=f"msq{li}{which}")
                nc.vector.tensor_tensor(msq[:, 0:w], mu[:, 0:w], mu[:, 0:w],
                                        mybir.AluOpType.mult)
                nc.vector.tensor_tensor(var[:, 0:w], var[:, 0:w], msq[:, 0:w],
                                        mybir.AluOpType.subtract)
                srt = sp.tile([1, RPC], F32, tag="lnsrt", name=f"srt{li}{which}")
                nc.scalar.activation(srt[:, 0:w], var[:, 0:w],
                                     mybir.ActivationFunctionType.Sqrt,
                                     bias=eps[:])
                rs = sp.tile([1, RPC], F32, tag="lnrstd", name=f"rst{li}{which}")
                nc.vector.reciprocal_approx_fast(rs[:, 0:w], srt[:, 0:w])
                rb = psA.tile([128, RPC], F32, name=f"lnA{li}{which}", tag="A")
                nc.tensor.matmul(rb[:, 0:w], ones_r[:], rs[:, 0:w],
                                 start=True, stop=True)
                rsS = tp.tile([128, RPC], F32, tag="lnrsS", name=f"rsS{li}{which}")
                nc.vector.tensor_copy(rsS[:, c0:RPC], rb[:, 0:w])
                qmr = rstdT = None
                if mi is not None:
                    qmr = tp.tile([128, RPC], F32, tag="lnqmr",
                                  name=f"qmr{li}{which}")
                    nc.vector.tensor_tensor(qmr[:, c0:RPC], rsS[:, c0:RPC],
                                            masksB[:, mi, c0:RPC],
                                            mybir.AluOpType.mult)
                    rstdT = sp.tile([128, 2], F32, tag="lnrT", name=f"rT{li}{which}")
                    for kb in range(2):
                        tpp = psA.tile([128, 1], F32, name=f"rT{li}{which}{kb}",
                                       tag="A")
                        nc.tensor.transpose(
                            tpp[:], rs[:, 128 * kb : 128 * (kb + 1)], one11[:])
                        nc.vector.tensor_copy(rstdT[:, kb : kb + 1], tpp[:])
                return mu16, rsS, qmr, rstdT

            def qproj(li, cs_t, mu16, qmr, c0=0):
                w = RPC - c0
                for j in range(2):
                    ch = wp.tile([128, 8, 512], BF16, tag="wc", name=f"wq{li}{j}")
                    nc.sync.dma_start(ch[:], wqkvc.ap()[li, j])
                    for mm in range(4):
                        mt = 4 * j + mm
                        ps = psA.tile([128, RPC], F32, name=f"q{li}{mt}", tag="A")
                        for a in range(8):
                            nc.tensor.matmul(ps[:, 0:w],
                                             ch[:, a, 128 * mm : 128 * (mm + 1)],
                                             xb[:, a, c0:RPC],
                                             start=(a == 0), stop=False)
                        nc.tensor.matmul(ps[:, 0:w],
                                         cs_t[:, 128 * mt : 128 * (mt + 1)],
                                         mu16[:, c0:RPC],
                                         start=False, stop=True)
                        nc.vector.tensor_tensor(QT[:, mt, c0:RPC], ps[:, 0:w],
                                                qmr[:, c0:RPC],
                                                mybir.AluOpType.mult)

            RG4 = [[0, 1, 2, 3], [4, 5, 6, 7]]

            def kprojA(li, cs_t, mu16, rsS):
                """K projection for my 256 rows; issues the K AllGather."""
                for j in range(2):
                    ch = wp.tile([128, 8, 512], BF16, tag="wc", name=f"wk{li}{j}")
                    nc.sync.dma_start(ch[:], wqkvc.ap()[li, 2 + j])
                    for mm in range(4):
                        mt = 4 * j + mm
                        ps = psA.tile([128, RPC], F32, name=f"k{li}{mt}", tag="A")
                        for a in range(8):
                            nc.tensor.matmul(ps[:],
                                             ch[:, a, 128 * mm : 128 * (mm + 1)],
                                             xb[:, a, :],
                                             start=(a == 0), stop=False)
                        nc.tensor.matmul(
                            ps[:], cs_t[:, D + 128 * mt : D + 128 * (mt + 1)],
                            mu16[:], start=False, stop=True)
                        nc.vector.tensor_tensor(KTst[:, mt, :], ps[:], rsS[:],
                                                mybir.AluOpType.mult)
                k_in = dp.tile([128, KCH], FP8, tag="kin", name=f"kin{li}")
                k_out = dp.tile([NG, 128, KCH], FP8, tag="kout", name=f"kout{li}")
                nc.sync.dma_start(
                    k_in[:].rearrange("p (a k) -> p a k", a=8), KTst[:])
                nc.gpsimd.collective_compute(
                    "AllGather", mybir.AluOpType.bypass, replica_groups=RG4,
                    ins=[k_in.opt()], outs=[k_out.opt()])
                return k_out

            def vprojA(li, cs_t, mu16, rstdT):
                """V projection for my 256 rows; issues the V AllGather."""
                for j in range(2):
                    ch = wp.tile([128, 8, 512], BF16, tag="wc", name=f"wv{li}{j}")
                    nc.sync.dma_start(ch[:], wqkvc.ap()[li, 4 + j])
                    for kb in range(2):
                        ps = psA.tile([128, 512], F32, name=f"v{li}{j}{kb}", tag="A")
                        for a in range(8):
                            nc.tensor.matmul(ps[:], xb[:, a, 128 * kb : 128 * (kb + 1)],
                                             ch[:, a, :],
                                             start=(a == 0), stop=False)
                        nc.tensor.matmul(
                            ps[:], mu16[:, 128 * kb : 128 * (kb + 1)],
                            cs_t[:, 2 * D + 512 * j : 2 * D + 512 * (j + 1)],
                            start=False, stop=True)
                        vsc = tp.tile([128, 512], F32, tag="vsc",
                                      name=f"vsc{li}{j}{kb}")
                        nc.vector.tensor_scalar_mul(vsc[:], ps[:],
                                                    rstdT[:, kb : kb + 1])
                        nc.vector.tensor_copy(
                            Vst[:, kb, 8 * j : 8 * (j + 1), 0:DH],
                            vsc[:].rearrange("p (h d) -> p h d", h=8))
                v_in = dp.tile([128, VCH], FP8, tag="vin", name=f"vin{li}")
                v_out = dp.tile([NG, 128, VCH], FP8, tag="vout", name=f"vout{li}")
                nc.sync.dma_start(
                    v_in[:].rearrange("p (b h d) -> p b h d", b=2, h=H), Vst[:])
                nc.gpsimd.collective_compute(
                    "AllGather", mybir.AluOpType.bypass, replica_groups=RG4,
                    ins=[v_in.opt()], outs=[v_out.opt()])
                return v_out

            def load_K(src, off=0):
                for r in range(NG):
                    nc.sync.dma_start(
                        Kfull[:, :, RPC * r : RPC * (r + 1)],
                        src[r][:, off : off + KCH].rearrange("p (a k) -> p a k",
                                                             a=8))

            def load_V(src, off=0):
                for r in range(NG):
                    nc.sync.dma_start(
                        Vfull[:, 2 * r : 2 * r + 2, :, :],
                        src[r][:, off : off + VCH].rearrange(
                            "p (b h d) -> p b h d", b=2, h=H))

            def attn(li, c0=0):
                w = RPC - c0
                for hp in range(8):          # head pair (2hp, 2hp+1), a-chunk hp
                    # one E tile per (z, half), each written by a single exp,
                    # so AV matmuls fire as soon as their chunk's exp lands.
                    Ez = [[tp.tile([128, 4, RPC], FP8, tag="E", bufs=8,
                                   name=f"E{li}{hp}{z}{half}")
                           for half in range(2)] for z in range(2)]
                    for half in range(2):
                        Scz = [psS.tile([128, 4, RPC], F32,
                                        name=f"sc{li}{hp}{half}{z}", tag="S")
                               for z in range(2)]
                        for i in range(4):
                            kb = 4 * half + i
                            for z in range(2):
                                po = 64 * z
                                nc.tensor.matmul(
                                    Scz[z][:, i, 0:w],
                                    Kfull[po : po + 64, hp,
                                          128 * kb : 128 * (kb + 1)],
                                    QT[po : po + 64, hp, c0:RPC],
                                    start=True, stop=True)
                        for z in range(2):
                            nc.scalar.activation(
                                Ez[z][half][:, :, 0:w],
                                Scz[z][:, :, 0:w],
                                mybir.ActivationFunctionType.Exp)
                    den = tp.tile([1, 2 * RPC], F32, tag="den", name=f"den{li}{hp}")
                    for z in range(2):
                        h = 2 * hp + z
                        po = 64 * z
                        O = psO.tile([DH + 1, RPC], F32, name=f"av{li}{h}", tag="O")
                        for kb in range(8):
                            nc.tensor.matmul(O[:, 0:w], Vfull[:, kb, h, :],
                                             Ez[z][kb // 4][:, kb % 4, 0:w],
                                             start=(kb == 0), stop=(kb == 7))
                        # custom-DVE reciprocal misreads PSUM inputs on HW:
                        # stage the denominator row through SBUF first.
                        nc.vector.tensor_copy(den[:, z * RPC : z * RPC + w],
                                              O[DH : DH + 1, 0:w])
                        rs = sp.tile([1, RPC], F32, tag="rs", name=f"rs{li}{h}")
                        nc.vector.reciprocal_approx_fast(
                            rs[:, 0:w], den[:, z * RPC : z * RPC + w])
                        bc = psA.tile([64, RPC], F32, name=f"nb{li}{h}", tag="A")
                        nc.tensor.matmul(bc[:, 0:w], ones_r[:, 0:64], rs[:, 0:w],
                                         start=True, stop=True)
                        rsb = sp.tile([64, RPC], F32, tag="rsb", name=f"rsb{li}{h}")
                        nc.vector.tensor_copy(rsb[:, 0:w], bc[:, 0:w])
                        nc.vector.tensor_tensor(OT[po : po + 64, hp, c0:RPC],
                                                O[0:DH, 0:w], rsb[:, 0:w],
                                                mybir.AluOpType.mult)

            def wo_add(li, c0=0):
                w = RPC - c0
                for j in range(2):
                    ch = wp.tile([128, 8, 512], BF16, tag="wc", name=f"woc{li}{j}")
                    nc.sync.dma_start(ch[:], woc.ap()[li, j])
                    for mm in range(4):
                        m = 4 * j + mm
                        ps = psA.tile([128, RPC], F32, name=f"y{li}{m}", tag="A")
                        for a in range(8):
                            nc.tensor.matmul(ps[:, 0:w],
                                             ch[:, a, 128 * mm : 128 * (mm + 1)],
                                             OT[:, a, c0:RPC],
                                             start=(a == 0), stop=(a == 7))
                        xs = xT[:, m, c0:RPC]
                        nc.vector.tensor_tensor(xs, ps[:, 0:w], xs,
                                                mybir.AluOpType.add)
                        nc.vector.tensor_copy(xb[:, m, c0:RPC], xs)

            def ffn(li, cs1_t, mu16, rsS, c0=0):
                w = RPC - c0
                for j in range(8):
                    ch = wp.tile([128, 8, 512], BF16, tag="wc", name=f"w1c{li}{j}")
                    nc.sync.dma_start(ch[:], w1c.ap()[li, j])
                    for mm in range(4):
                        m1 = 4 * j + mm
                        ps = psA.tile([128, RPC], F32, name=f"u{li}{m1}", tag="A")
                        for a in range(8):
                            nc.tensor.matmul(ps[:, 0:w],
                                             ch[:, a, 128 * mm : 128 * (mm + 1)],
                                             xb[:, a, c0:RPC],
                                             start=(a == 0), stop=False)
                        nc.tensor.matmul(ps[:, 0:w],
                                         cs1_t[:, 128 * m1 : 128 * (m1 + 1)],
                                         mu16[:, c0:RPC], start=False, stop=True)
                        us = tp.tile([128, RPC], BF16, tag="us",
                                     name=f"us{li}{m1}")
                        nc.vector.tensor_tensor(us[:, 0:w], ps[:, 0:w],
                                                rsS[:, c0:RPC],
                                                mybir.AluOpType.mult)
                        nc.scalar.activation(G[:, m1, c0:RPC], us[:, 0:w],
                                             mybir.ActivationFunctionType.Gelu)
                for m2 in range(8):
                    ch = wp.tile([128, 32, 128], BF16, tag="wc", name=f"w2c{li}{m2}")
                    nc.sync.dma_start(ch[:], w2c.ap()[li, m2])
                    ps = psA.tile([128, RPC], F32, name=f"z{li}{m2}", tag="A")
                    for ka in range(32):
                        nc.tensor.matmul(ps[:, 0:w], ch[:, ka, :],
                                         G[:, ka, c0:RPC],
                                         start=(ka == 0), stop=(ka == 31))
                    xs = xT[:, m2, c0:RPC]
                    nc.vector.tensor_tensor(xs, ps[:, 0:w], xs, mybir.AluOpType.add)
                    nc.vector.tensor_copy(xb[:, m2, c0:RPC], xs)

            def load_cs(li):
                cs_t = sp.tile([1, 3 * D], BF16, tag="cs", name=f"cs{li}", bufs=1)
                nc.sync.dma_start(cs_t[:], csqkv.ap()[li])
                cs1_t = sp.tile([1, DFF], BF16, tag="cs1", name=f"cs1{li}", bufs=1)
                nc.sync.dma_start(cs1_t[:], csw1.ap()[li])
                return cs_t, cs1_t

            # ---- prologue: layer-0 LN1 + Q proj; K/V comes precomputed ----
            cs_t, cs1_t = load_cs(0)
            mu16, rsS, qmr, rstdT = ln_stats(0, "p", mi=0)
            qproj(0, cs_t, mu16, qmr)
            load_K(kv0.ap())
            load_V(kv0.ap(), off=KCH)

            ag = None
            for li in range(n_layers):
                last = li == n_layers - 1
                c0 = 128 if last else 0
                mi_next = 0 if li + 1 < NL else 1
                with nc.named_scope(f"L{li}_attn"):
                    attn(li, c0)
                with nc.named_scope(f"L{li}_wo"):
                    wo_add(li, c0)
                    mu16n, rsSn, _, _ = ln_stats(li, "n", c0=c0)
                with nc.named_scope(f"L{li}_ffn"):
                    ffn(li, cs1_t, mu16n, rsSn, c0)
                if not last:
                    with nc.named_scope(f"L{li}_kv"):
                        cs_t, cs1_t = load_cs(li + 1)
                        mu16, rsS, qmr, rstdT = ln_stats(li + 1, "p", mi=mi_next)
                        agk = kprojA(li + 1, cs_t, mu16, rsS)
                        agv = vprojA(li + 1, cs_t, mu16, rstdT)
                        qproj(li + 1, cs_t, mu16, qmr,
                              128 if li + 1 == n_layers - 1 else 0)
                        load_K(agk)
                        load_V(agv)
                else:
                    with nc.named_scope("xag"):
                        x_in = dp.tile([128, 1024], BF16, tag="xin", name="xin")
                        nc.sync.dma_start(
                            x_in[:].rearrange("p (a k) -> p a k", a=8),
                            xb[:, :, 128:RPC])
                        agx = dp.tile([NC, 128, 1024], BF16, tag="xout",
                                      name="xout", addr_space="Shared")
                        nc.gpsimd.collective_compute(
                            "AllGather", mybir.AluOpType.bypass,
                            replica_groups=[list(range(NC))],
                            ins=[x_in.opt()], outs=[agx.opt()])

            # ---- target logits: local mask rows vs their target embeddings ----
            with nc.named_scope("tlogit"):
                Et = pp.tile([128, 8, 128], BF16, name="Et")
                nc.sync.dma_start(Et[:], etT.ap())
                tlp = psA.tile([1, 128], F32, name="tlp", tag="A")
                for a in range(8):
                    P = tp.tile([128, 128], F32, tag="P", name=f"P{a}")
                    nc.vector.tensor_tensor(P[:], xb[:, a, 128:RPC], Et[:, a, :],
                                            mybir.AluOpType.mult)
                    nc.tensor.matmul(tlp[:], ones[:], P[:],
                                     start=(a == 0), stop=(a == 7))
                tl_sb = sp.tile([1, 128], F32, tag="tlsb", name="tlsb", bufs=1)
                nc.vector.tensor_copy(tl_sb[:], tlp[:])
                nc.sync.dma_start(tlogit_o.ap(), tl_sb[:])

            # ---- unembedding (vocab-sharded) ----
            with nc.named_scope("unembed"):
                NV = 500
                xfull = pp.tile([128, 8, 8, 128], BF16, name="xfull")
                se_parts = pp.tile([128, 8, 8], F32, name="separts")
                for t in range(8):
                    nc.sync.dma_start(
                        xfull[:, :, t, :],
                        agx[t].rearrange("p (a k) -> p a k", a=8))
                for n in range(8):
                    ch = ep.tile([128, 8, NV], BF16, tag="emb", name=f"ec{n}")
                    nc.sync.dma_start(ch[:], embc.ap()[n])
                    for t in range(8):
                        ps = psA.tile([128, NV], F32, name=f"lg{n}{t}", tag="A")
                        for a in range(8):
                            nc.tensor.matmul(ps[:], xfull[:, a, t, :], ch[:, a, :],
                                             start=(a == 0), stop=(a == 7))
                        Esc = ep.tile([128, NV], BF16, tag="esc", name=f"esc{n}{t}")
                        nc.scalar.activation(Esc[:], ps[:],
                                             mybir.ActivationFunctionType.Exp,
                                             accum_out=se_parts[:, n, t : t + 1])
                se = sp.tile([128, 8], F32, tag="se", name="se")
                for t in range(8):
                    nc.vector.reduce_sum(se[:, t : t + 1], se_parts[:, :, t],
                                         axis=mybir.AxisListType.X)
                nc.sync.dma_start(sumexp_o.ap(), se[:])

    nc.finalize()
    return nc


def _rows_of(j):
    tok = np.arange(128 * j, 128 * j + 128)
    return np.concatenate([tok, 512 + tok])


def _qmask_rows(rows):
    """[2, 256] keep-masks (block, ar) for the given global row ids."""
    m = np.ones((2, len(rows)), np.float32)
    for i, g in enumerate(rows):
        if T - 2 * CSL <= g < T:
            m[0, i] = 0.0
        if g == T - 1 or (g >= T and (g - T) % CSL == CSL - 1):
            m[1, i] = 0.0
    return m


def _prep(inputs):
    """Host-side input prep -> per-core in_maps."""
    f = {k: np.asarray(v) for k, v in inputs.items()}
    tok_ids = f["tok_ids"].astype(np.int64)
    tok_emb = f["tok_emb"].astype(np.float32)
    pos_emb = f["pos_emb"].astype(np.float32)
    mask_tokens = f["mask_tokens"].astype(np.float32)

    # x0 [B, S, D]
    x0 = np.empty((B, S, D), np.float32)
    for b in range(B):
        x0[b, :T] = tok_emb[tok_ids[b]]
        x0[b, T:] = np.tile(mask_tokens[0], (T // CSL, 1))
    x0 += pos_emb[np.arange(S) % T][None]

    def stack(name):
        return np.concatenate([f["b_" + name], f["d_" + name]], axis=0)

    wqkv = stack("wqkv").astype(np.float32)
    wqkv_s = wqkv.copy()
    wqkv_s[:, :, :D] /= np.sqrt(DH)
    wo_s = stack("wo").astype(np.float32)
    w1_s = stack("w1").astype(np.float32)
    w2_s = stack("w2").astype(np.float32)

    # negated column sums for the rank-1 LN-mean correction
    csqkv_h = np.ascontiguousarray(
        (-wqkv_s.sum(axis=1))[:, None, :]).astype(BF)     # [6, 1, 3072]
    csw1_h = np.ascontiguousarray(
        (-w1_s.sum(axis=1))[:, None, :]).astype(BF)       # [6, 1, 4096]

    def chunk_cols(w, ncols):
        # [NL, D, M] -> [NL, M//ncols, 128, 8, ncols]  (row d = a*128+p)
        nl, d, m = w.shape
        out = w.reshape(nl, 8, 128, m // ncols, ncols).transpose(0, 3, 2, 1, 4)
        return np.ascontiguousarray(out).astype(BF)

    wqkvc = chunk_cols(wqkv_s, 512)                      # [6, 6, 128, 8, 512]
    woc = chunk_cols(wo_s, 512)                          # [6, 2, 128, 8, 512]
    w1cc = chunk_cols(w1_s, 512)                         # [6, 8, 128, 8, 512]
    # w2: [NL, DFF, D] -> [6, 8 m2, 128 p, 32 ka, 128 q]
    w2r = w2_s.reshape(NLAYERS, 32, 128, 8, 128)         # [nl, ka, p, m2, q]
    w2cc = np.ascontiguousarray(
        w2r.transpose(0, 3, 2, 1, 4)).astype(BF)

    # layer-0 K/V per elem (host precompute; LN g=1 b=0)
    mu = x0.mean(-1, keepdims=True)
    var = x0.var(-1, keepdims=True)
    h0 = (x0 - mu) / np.sqrt(var + 1e-5)
    K0 = h0 @ wqkv[0, :, D : 2 * D]                      # [B, S, D] (unscaled)
    V0 = h0 @ wqkv[0, :, 2 * D : 3 * D]

    # per-core targets + their embeddings in local row order
    tgt = np.full(NC * 128, -1, np.int64)
    etTs = []
    for c in range(NC):
        e, j = c // 4, c % 4
        et_f = np.zeros((128, D), np.float32)
        for p in range(128):
            g = T + 128 * j + p
            if g >= T + 1:
                tid = tok_ids[e, g - T - 1]
                tgt[128 * c + p] = tid
                et_f[p] = tok_emb[tid]
        # local etT layout [128 dp, 8 a, 128 p]
        etTs.append(np.ascontiguousarray(
            et_f.T.reshape(8, 128, 128).transpose(1, 0, 2)).astype(BF))

    embT = tok_emb.T.astype(np.float32)                  # [D, V]

    in_maps = []
    for c in range(NC):
        e, j = c // 4, c % 4
        rows = _rows_of(j)
        x0T = np.ascontiguousarray(x0[e][rows].T)        # [D, 256] f32

        kv0c = np.empty((NG, 128, KVFREE), F8)
        for r in range(NG):
            rr = _rows_of(r)
            kT = K0[e][rr].T.reshape(8, 128, RPC).transpose(1, 0, 2)  # [p, a, k]
            kv0c[r, :, 0:KCH] = kT.reshape(128, KCH).astype(F8)
            vpart = np.empty((128, 2, H, DH + 1), np.float32)
            vloc = V0[e][rr].reshape(2, 128, H, DH).transpose(1, 0, 2, 3)
            vpart[:, :, :, 0:DH] = vloc
            vpart[:, :, :, DH] = 1.0
            kv0c[r, :, KCH:KVFREE] = vpart.reshape(128, VCH).astype(F8)

        # vocab shard chunks [n, p, a, v] = emb_sh[a*128+p, 500n+v]
        emb_sh = embT[:, VS * c : VS * (c + 1)]          # [D, 4000]
        embcc = np.ascontiguousarray(
            emb_sh.reshape(8, 128, 8, 500).transpose(2, 1, 0, 3)).astype(BF)

        in_maps.append({
            "x0": x0T,
            "kv0": kv0c,
            "wqkvc": wqkvc, "woc": woc, "w1c": w1cc, "w2c": w2cc,
            "csqkv": csqkv_h, "csw1": csw1_h,
            "qm": _qmask_rows(rows),
            "embc": embcc,
            "etT": etTs[c],
        })
    return in_maps, tgt


def _combine(results, tgt):
    se = np.zeros((NC * 128,), np.float64)
    for c in range(NC):
        # sumexp out [128 p, 8 t] -> flat tk = t*128 + p
        se += results[c]["sumexp"].astype(np.float64).T.reshape(-1)
    tl = np.concatenate(
        [results[c]["tlogit"].astype(np.float64).reshape(-1) for c in range(NC)])
    valid = tgt >= 0
    lse = np.log(se[valid])
    return np.float32(np.mean(lse - tl[valid]))


def kernel(**inputs):
    if "nc" not in _CACHE:
        _CACHE["nc"] = _build_nc()
    nc = _CACHE["nc"]
    in_maps, tgt = _prep(inputs)
    res = run_bass_kernel_spmd(nc, in_maps, core_ids=list(range(NC)))
    return _combine(res.results, tgt)


# revision 23
# speedup vs baseline: 1.0873x; 1.0873x over previous
"""BlockNTP transformer forward + cross-entropy loss on 8 trn2 NeuronCores.

v3: LayerNorm folded into matmul consumers so the PE never stalls on the LN
chain.  For row q with mean mu[q], rstd[q]:
  W^T h = (W^T x - colsum(W) * mu[q]) * rstd[q]
The colsum term is ONE extra rank-1 accumulation matmul per psum tile
(lhsT = -colsum chunk [1,128], rhs = mu16 [1,rows]); the rstd scale folds
into the existing psum-consumer DVE ops (Q mask-mult, K mult, V per-partition
tensor_scalar, w1 pre-gelu mult).  The PE starts qkv/w1 matmul groups right
after the residual adds; only the cheap DVE consumers wait on the rstd chain.

Sharding: cores 0-3 own batch elem 0, cores 4-7 elem 1; core (e, j) owns
256 rows (token rows [128j,128j+128) + mask rows 512+[128j,128j+128)).
Weights replicated (streamed bf16).  Per layer ONE fp8e4m3 K/V AllGather
(two 4-rank groups).  Layer-0 K/V precomputed host-side.  Masks degenerate
to all-or-nothing rows -> zeroed Q rows give exact uniform softmax.
Attention runs head PAIRS on disjoint PE row-groups; softmax denominators
ride in V's ones-row; normalization uses reciprocal_approx_fast via an SBUF
staging copy (custom DVE ops misread PSUM).  Q/K/V/E all fp8e4m3.
Target logits are computed locally per core; only sum-exp needs the final
8-rank x AllGather.
"""
import numpy as np
import ml_dtypes

import concourse.bass as bass
import concourse.mybir as mybir
import concourse.tile as tile
from concourse import bacc
from concourse.bass_utils import run_bass_kernel_spmd

B, T = 2, 512
D, H, DFF = 1024, 16, 4096
V, CSL = 32000, 16
NL, NDL = 4, 2
NLAYERS = NL + NDL
DH = D // H
S = 2 * T                    # 1024 rows per batch elem
NC = 8                       # cores
NG = 4                       # cores per elem
RPC = 256                    # rows per core (128 token + 128 mask)
VS = V // NC                 # 4000 vocab per core
F32 = mybir.dt.float32
BF16 = mybir.dt.bfloat16
FP8 = mybir.dt.float8e4
BF = ml_dtypes.bfloat16
F8 = ml_dtypes.float8_e4m3

KCH = 8 * RPC                # 2048: K^T part free elems per chunk
VCH = 2 * H * (DH + 1)       # 2080: V part free elems per chunk
KVFREE = KCH + VCH           # 4128

_CACHE = {}


def _build_nc(n_layers=NLAYERS):
    nc = bacc.Bacc("TRN2", target_bir_lowering=False, debug=False, num_devices=NC)

    x0 = nc.dram_tensor("x0", [D, RPC], F32, kind="ExternalInput")
    kv0 = nc.dram_tensor("kv0", [NG, 128, KVFREE], FP8, kind="ExternalInput")
    wqkvc = nc.dram_tensor("wqkvc", [NLAYERS, 6, 128, 8, 512], BF16,
                           kind="ExternalInput")
    woc = nc.dram_tensor("woc", [NLAYERS, 2, 128, 8, 512], BF16,
                         kind="ExternalInput")
    w1c = nc.dram_tensor("w1c", [NLAYERS, 8, 128, 8, 512], BF16,
                         kind="ExternalInput")
    w2c = nc.dram_tensor("w2c", [NLAYERS, 8, 128, 16, 2, 128], FP8,
                         kind="ExternalInput")
    csqkv = nc.dram_tensor("csqkv", [NLAYERS, 1, 3 * D], BF16,
                           kind="ExternalInput")
    csw1 = nc.dram_tensor("csw1", [NLAYERS, 1, DFF], BF16, kind="ExternalInput")
    qm = nc.dram_tensor("qm", [2, RPC], F32, kind="ExternalInput")
    embc = nc.dram_tensor("embc", [8, 128, 8, 500], BF16, kind="ExternalInput")
    etT = nc.dram_tensor("etT", [128, 8, 128], BF16, kind="ExternalInput")
    sumexp_o = nc.dram_tensor("sumexp", [128, 8], F32, kind="ExternalOutput")
    tlogit_o = nc.dram_tensor("tlogit", [1, 128], F32, kind="ExternalOutput")

    with tile.TileContext(nc) as tc:
        with (
            tc.tile_pool(name="persist", bufs=1) as pp,
            tc.tile_pool(name="wpool", bufs=4) as wp,
            tc.tile_pool(name="epool", bufs=3) as ep,
            tc.tile_pool(name="tmp", bufs=2) as tp,
            tc.tile_pool(name="small", bufs=2) as sp,
            tc.tile_pool(name="psS", bufs=2, space="PSUM") as psS,
            tc.tile_pool(name="psA", bufs=2, space="PSUM") as psA,
            tc.tile_pool(name="psO", bufs=2, space="PSUM") as psO,
            tc.tile_pool(name="dram", bufs=2, space="DRAM") as dp,
        ):
            xT = pp.tile([128, 8, RPC], F32, name="xT")
            xb = pp.tile([128, 8, RPC], BF16, name="xb")
            QT = pp.tile([128, 8, RPC], FP8, name="QT")
            Kfull = pp.tile([128, 8, S], FP8, name="Kfull")
            Vfull = pp.tile([128, 8, H, DH + 1], FP8, name="Vfull")
            OT = pp.tile([128, 8, RPC], BF16, name="OT")
            G = pp.tile([128, 32, RPC], FP8, name="G")
            KTst = pp.tile([128, 8, RPC], FP8, name="KTst")
            Vst = pp.tile([128, 2, H, DH + 1], FP8, name="Vst")
            ones = pp.tile([128, 1], F32, name="ones")
            nc.vector.memset(ones[:], 1.0)
            ones16 = pp.tile([128, 1], BF16, name="ones16")
            nc.vector.memset(ones16[:], 1.0)
            ones_r = pp.tile([1, 128], F32, name="ones_r")
            nc.vector.memset(ones_r[:], 1.0)
            one11 = pp.tile([1, 1], F32, name="one11")
            nc.vector.memset(one11[:], 1.0)
            eps = pp.tile([1, 1], F32, name="eps")
            nc.vector.memset(eps[:], 1e-5)
            nc.vector.memset(Vst[:, :, :, DH : DH + 1], 1.0)
            masks = pp.tile([1, 2 * RPC], F32, name="masks")
            nc.sync.dma_start(masks[:, 0:RPC], qm.ap()[0:1, :])
            nc.sync.dma_start(masks[:, RPC : 2 * RPC], qm.ap()[1:2, :])
            masksB = pp.tile([128, 2, RPC], F32, name="masksB")
            for mi in range(2):
                mb = psA.tile([128, RPC], F32, name=f"mb{mi}", tag="A")
                nc.tensor.matmul(mb[:], ones_r[:],
                                 masks[:, mi * RPC : (mi + 1) * RPC],
                                 start=True, stop=True)
                nc.vector.tensor_copy(masksB[:, mi, :], mb[:])

            nc.sync.dma_start(xT[:], x0.ap().rearrange("(a p) c -> p a c", p=128))
            for a in range(8):
                nc.vector.tensor_copy(xb[:, a, :], xT[:, a, :])

            def ln_stats(li, which, mi=None, c0=0):
                """LN stats over D (partitions) of xb cols [c0,RPC)."""
                w = RPC - c0
                ps1 = psA.tile([1, RPC], F32, name=f"s1_{li}{which}", tag="A")
                ps2 = psA.tile([1, RPC], F32, name=f"s2_{li}{which}", tag="A")
                sqf = tp.tile([128, 8, RPC], F32, tag="lnsq", name=f"sq{li}{which}")
                for a in range(8):
                    xs = xb[:, a, c0:RPC]
                    nc.vector.tensor_tensor(sqf[:, a, 0:w], xs, xs,
                                            mybir.AluOpType.mult)
                for a in range(8):
                    nc.tensor.matmul(ps1[:, 0:w], ones16[:], xb[:, a, c0:RPC],
                                     start=(a == 0), stop=(a == 7))
                for a in range(8):
                    nc.tensor.matmul(ps2[:, 0:w], ones[:], sqf[:, a, 0:w],
                                     start=(a == 0), stop=(a == 7))
                mu = sp.tile([1, RPC], F32, tag="lnmu", name=f"mu{li}{which}")
                var = sp.tile([1, RPC], F32, tag="lnvar", name=f"var{li}{which}")
                nc.vector.tensor_scalar_mul(mu[:, 0:w], ps1[:, 0:w], 1.0 / D)
                nc.vector.tensor_scalar_mul(var[:, 0:w], ps2[:, 0:w], 1.0 / D)
                mu16 = sp.tile([1, RPC], BF16, tag="lnmu16", name=f"m16{li}{which}")
                nc.vector.tensor_copy(mu16[:, c0:RPC], mu[:, 0:w])
                msq = sp.tile([1, RPC], F32, tag="lnmsq", name=f"msq{li}{which}")
                nc.vector.tensor_tensor(msq[:, 0:w], mu[:, 0:w], mu[:, 0:w],
                                        mybir.AluOpType.mult)
                nc.vector.tensor_tensor(var[:, 0:w], var[:, 0:w], msq[:, 0:w],
                                        mybir.AluOpType.subtract)
                srt = sp.tile([1, RPC], F32, tag="lnsrt", name=f"srt{li}{which}")
                nc.scalar.activation(srt[:, 0:w], var[:, 0:w],
                                     mybir.ActivationFunctionType.Sqrt,
                                     bias=eps[:])
                rs = sp.tile([1, RPC], F32, tag="lnrstd", name=f"rst{li}{which}")
                nc.vector.reciprocal_approx_fast(rs[:, 0:w], srt[:, 0:w])
                rb = psA.tile([128, RPC], F32, name=f"lnA{li}{which}", tag="A")
                nc.tensor.matmul(rb[:, 0:w], ones_r[:], rs[:, 0:w],
                                 start=True, stop=True)
                rsS = tp.tile([128, RPC], F32, tag="lnrsS", name=f"rsS{li}{which}")
                nc.vector.tensor_copy(rsS[:, c0:RPC], rb[:, 0:w])
                qmr = rstdT = None
                if mi is not None:
                    qmr = tp.tile([128, RPC], F32, tag="lnqmr",
                                  name=f"qmr{li}{which}")
                    nc.vector.tensor_tensor(qmr[:, c0:RPC], rsS[:, c0:RPC],
                                            masksB[:, mi, c0:RPC],
                                            mybir.AluOpType.mult)
                    rstdT = sp.tile([128, 2], F32, tag="lnrT", name=f"rT{li}{which}")
                    for kb in range(2):
                        tpp = psA.tile([128, 1], F32, name=f"rT{li}{which}{kb}",
                                       tag="A")
                        nc.tensor.transpose(
                            tpp[:], rs[:, 128 * kb : 128 * (kb + 1)], one11[:])
                        nc.vector.tensor_copy(rstdT[:, kb : kb + 1], tpp[:])
                return mu16, rsS, qmr, rstdT

            def qproj(li, cs_t, mu16, qmr, c0=0):
                w = RPC - c0
                for j in range(2):
                    ch = wp.tile([128, 8, 512], BF16, tag="wc", name=f"wq{li}{j}")
                    nc.sync.dma_start(ch[:], wqkvc.ap()[li, j])
                    for mm in range(4):
                        mt = 4 * j + mm
                        ps = psA.tile([128, RPC], F32, name=f"q{li}{mt}", tag="A")
                        for a in range(8):
                            nc.tensor.matmul(ps[:, 0:w],
                                             ch[:, a, 128 * mm : 128 * (mm + 1)],
                                             xb[:, a, c0:RPC],
                                             start=(a == 0), stop=False)
                        nc.tensor.matmul(ps[:, 0:w],
                                         cs_t[:, 128 * mt : 128 * (mt + 1)],
                                         mu16[:, c0:RPC],
                                         start=False, stop=True)
                        nc.vector.tensor_tensor(QT[:, mt, c0:RPC], ps[:, 0:w],
                                                qmr[:, c0:RPC],
                                                mybir.AluOpType.mult)

            RG4 = [[0, 1, 2, 3], [4, 5, 6, 7]]

            def kprojA(li, cs_t, mu16, rsS):
                """K projection for my 256 rows; issues the K AllGather."""
                for j in range(2):
                    ch = wp.tile([128, 8, 512], BF16, tag="wc", name=f"wk{li}{j}")
                    nc.sync.dma_start(ch[:], wqkvc.ap()[li, 2 + j])
                    for mm in range(4):
                        mt = 4 * j + mm
                        ps = psA.tile([128, RPC], F32, name=f"k{li}{mt}", tag="A")
                        for a in range(8):
                            nc.tensor.matmul(ps[:],
                                             ch[:, a, 128 * mm : 128 * (mm + 1)],
                                             xb[:, a, :],
                                             start=(a == 0), stop=False)
                        nc.tensor.matmul(
                            ps[:], cs_t[:, D + 128 * mt : D + 128 * (mt + 1)],
                            mu16[:], start=False, stop=True)
                        nc.vector.tensor_tensor(KTst[:, mt, :], ps[:], rsS[:],
                                                mybir.AluOpType.mult)
                k_in = dp.tile([128, KCH], FP8, tag="kin", name=f"kin{li}")
                k_out = dp.tile([NG, 128, KCH], FP8, tag="kout", name=f"kout{li}")
                nc.sync.dma_start(
                    k_in[:].rearrange("p (a k) -> p a k", a=8), KTst[:])
                nc.gpsimd.collective_compute(
                    "AllGather", mybir.AluOpType.bypass, replica_groups=RG4,
                    ins=[k_in.opt()], outs=[k_out.opt()])
                return k_out

            def vprojA(li, cs_t, mu16, rstdT):
                """V projection for my 256 rows; issues the V AllGather."""
                for j in range(2):
                    ch = wp.tile([128, 8, 512], BF16, tag="wc", name=f"wv{li}{j}")
                    nc.sync.dma_start(ch[:], wqkvc.ap()[li, 4 + j])
                    for kb in range(2):
                        ps = psA.tile([128, 512], F32, name=f"v{li}{j}{kb}", tag="A")
                        for a in range(8):
                            nc.tensor.matmul(ps[:], xb[:, a, 128 * kb : 128 * (kb + 1)],
                                             ch[:, a, :],
                                             start=(a == 0), stop=False)
                        nc.tensor.matmul(
                            ps[:], mu16[:, 128 * kb : 128 * (kb + 1)],
                            cs_t[:, 2 * D + 512 * j : 2 * D + 512 * (j + 1)],
                            start=False, stop=True)
                        vsc = tp.tile([128, 512], F32, tag="vsc",
                                      name=f"vsc{li}{j}{kb}")
                        nc.vector.tensor_scalar_mul(vsc[:], ps[:],
                                                    rstdT[:, kb : kb + 1])
                        nc.vector.tensor_copy(
                            Vst[:, kb, 8 * j : 8 * (j + 1), 0:DH],
                            vsc[:].rearrange("p (h d) -> p h d", h=8))
                v_in = dp.tile([128, VCH], FP8, tag="vin", name=f"vin{li}")
                v_out = dp.tile([NG, 128, VCH], FP8, tag="vout", name=f"vout{li}")
                nc.sync.dma_start(
                    v_in[:].rearrange("p (b h d) -> p b h d", b=2, h=H), Vst[:])
                nc.gpsimd.collective_compute(
                    "AllGather", mybir.AluOpType.bypass, replica_groups=RG4,
                    ins=[v_in.opt()], outs=[v_out.opt()])
                return v_out

            def load_K(src, off=0):
                for r in range(NG):
                    nc.sync.dma_start(
                        Kfull[:, :, RPC * r : RPC * (r + 1)],
                        src[r][:, off : off + KCH].rearrange("p (a k) -> p a k",
                                                             a=8))

            def load_V(src, off=0):
                for r in range(NG):
                    nc.sync.dma_start(
                        Vfull[:, 2 * r : 2 * r + 2, :, :],
                        src[r][:, off : off + VCH].rearrange(
                            "p (b h d) -> p b h d", b=2, h=H))

            def attn(li, c0=0):
                w = RPC - c0
                for hp in range(8):          # head pair (2hp, 2hp+1), a-chunk hp
                    # one E tile per (z, half), each written by a single exp,
                    # so AV matmuls fire as soon as their chunk's exp lands.
                    Ez = [[tp.tile([128, 4, RPC], FP8, tag="E", bufs=8,
                                   name=f"E{li}{hp}{z}{half}")
                           for half in range(2)] for z in range(2)]
                    for half in range(2):
                        Scz = [psS.tile([128, 4, RPC], F32,
                                        name=f"sc{li}{hp}{half}{z}", tag="S")
                               for z in range(2)]
                        for i in range(4):
                            kb = 4 * half + i
                            for z in range(2):
                                po = 64 * z
                                nc.tensor.matmul(
                                    Scz[z][:, i, 0:w],
                                    Kfull[po : po + 64, hp,
                                          128 * kb : 128 * (kb + 1)],
                                    QT[po : po + 64, hp, c0:RPC],
                                    start=True, stop=True)
                        for z in range(2):
                            nc.scalar.activation(
                                Ez[z][half][:, :, 0:w],
                                Scz[z][:, :, 0:w],
                                mybir.ActivationFunctionType.Exp)
                    den = tp.tile([1, 2 * RPC], F32, tag="den", name=f"den{li}{hp}")
                    for z in range(2):
                        h = 2 * hp + z
                        po = 64 * z
                        O = psO.tile([DH + 1, RPC], F32, name=f"av{li}{h}", tag="O")
                        for kb in range(8):
                            nc.tensor.matmul(O[:, 0:w], Vfull[:, kb, h, :],
                                             Ez[z][kb // 4][:, kb % 4, 0:w],
                                             start=(kb == 0), stop=(kb == 7))
                        # custom-DVE reciprocal misreads PSUM inputs on HW:
                        # stage the denominator row through SBUF first.
                        nc.vector.tensor_copy(den[:, z * RPC : z * RPC + w],
                                              O[DH : DH + 1, 0:w])
                        rs = sp.tile([1, RPC], F32, tag="rs", name=f"rs{li}{h}")
                        nc.vector.reciprocal_approx_fast(
                            rs[:, 0:w], den[:, z * RPC : z * RPC + w])
                        bc = psA.tile([64, RPC], F32, name=f"nb{li}{h}", tag="A")
                        nc.tensor.matmul(bc[:, 0:w], ones_r[:, 0:64], rs[:, 0:w],
                                         start=True, stop=True)
                        rsb = sp.tile([64, RPC], F32, tag="rsb", name=f"rsb{li}{h}")
                        nc.vector.tensor_copy(rsb[:, 0:w], bc[:, 0:w])
                        nc.vector.tensor_tensor(OT[po : po + 64, hp, c0:RPC],
                                                O[0:DH, 0:w], rsb[:, 0:w],
                                                mybir.AluOpType.mult)

            def wo_add(li, c0=0):
                w = RPC - c0
                for j in range(2):
                    ch = wp.tile([128, 8, 512], BF16, tag="wc", name=f"woc{li}{j}")
                    nc.sync.dma_start(ch[:], woc.ap()[li, j])
                    for mm in range(4):
                        m = 4 * j + mm
                        ps = psA.tile([128, RPC], F32, name=f"y{li}{m}", tag="A")
                        for a in range(8):
                            nc.tensor.matmul(ps[:, 0:w],
                                             ch[:, a, 128 * mm : 128 * (mm + 1)],
                                             OT[:, a, c0:RPC],
                                             start=(a == 0), stop=(a == 7))
                        xs = xT[:, m, c0:RPC]
                        nc.vector.tensor_tensor(xs, ps[:, 0:w], xs,
                                                mybir.AluOpType.add)
                        nc.vector.tensor_copy(xb[:, m, c0:RPC], xs)

            def ffn(li, cs1_t, mu16, rsS, c0=0):
                w = RPC - c0
                for j in range(8):
                    ch = wp.tile([128, 8, 512], BF16, tag="wc", name=f"w1c{li}{j}")
                    nc.sync.dma_start(ch[:], w1c.ap()[li, j])
                    for mm in range(4):
                        m1 = 4 * j + mm
                        ps = psA.tile([128, RPC], F32, name=f"u{li}{m1}", tag="A")
                        for a in range(8):
                            nc.tensor.matmul(ps[:, 0:w],
                                             ch[:, a, 128 * mm : 128 * (mm + 1)],
                                             xb[:, a, c0:RPC],
                                             start=(a == 0), stop=False)
                        nc.tensor.matmul(ps[:, 0:w],
                                         cs1_t[:, 128 * m1 : 128 * (m1 + 1)],
                                         mu16[:, c0:RPC], start=False, stop=True)
                        us = tp.tile([128, RPC], BF16, tag="us",
                                     name=f"us{li}{m1}")
                        nc.vector.tensor_tensor(us[:, 0:w], ps[:, 0:w],
                                                rsS[:, c0:RPC],
                                                mybir.AluOpType.mult)
                        nc.scalar.activation(G[:, m1, c0:RPC], us[:, 0:w],
                                             mybir.ActivationFunctionType.Gelu)
                for m2 in range(8):
                    ch = wp.tile([128, 16, 2, 128], FP8, tag="wc",
                                 name=f"w2c{li}{m2}")
                    nc.sync.dma_start(ch[:], w2c.ap()[li, m2])
                    ps = psA.tile([128, RPC], F32, name=f"z{li}{m2}", tag="A")
                    for kp in range(16):
                        nc.tensor.matmul(ps[:, 0:w], ch[:, kp, :, :],
                                         G[:, 2 * kp : 2 * kp + 2, c0:RPC],
                                         start=(kp == 0), stop=(kp == 15),
                                         perf_mode=mybir.MatmulPerfMode.DoubleRow)
                    xs = xT[:, m2, c0:RPC]
                    # w2 was scaled x32 on the host to clear fp8e4 subnormals
                    nc.vector.scalar_tensor_tensor(
                        xs, ps[:, 0:w], 1.0 / 32.0, xs,
                        mybir.AluOpType.mult, mybir.AluOpType.add)
                    nc.vector.tensor_copy(xb[:, m2, c0:RPC], xs)

            def load_cs(li):
                cs_t = sp.tile([1, 3 * D], BF16, tag="cs", name=f"cs{li}", bufs=1)
                nc.sync.dma_start(cs_t[:], csqkv.ap()[li])
                cs1_t = sp.tile([1, DFF], BF16, tag="cs1", name=f"cs1{li}", bufs=1)
                nc.sync.dma_start(cs1_t[:], csw1.ap()[li])
                return cs_t, cs1_t

            # ---- prologue: layer-0 LN1 + Q proj; K/V comes precomputed ----
            cs_t, cs1_t = load_cs(0)
            mu16, rsS, qmr, rstdT = ln_stats(0, "p", mi=0)
            qproj(0, cs_t, mu16, qmr)
            load_K(kv0.ap())
            load_V(kv0.ap(), off=KCH)

            ag = None
            for li in range(n_layers):
                last = li == n_layers - 1
                c0 = 128 if last else 0
                mi_next = 0 if li + 1 < NL else 1
                with nc.named_scope(f"L{li}_attn"):
                    attn(li, c0)
                with nc.named_scope(f"L{li}_wo"):
                    wo_add(li, c0)
                    mu16n, rsSn, _, _ = ln_stats(li, "n", c0=c0)
                with nc.named_scope(f"L{li}_ffn"):
                    ffn(li, cs1_t, mu16n, rsSn, c0)
                if not last:
                    with nc.named_scope(f"L{li}_kv"):
                        cs_t, cs1_t = load_cs(li + 1)
                        mu16, rsS, qmr, rstdT = ln_stats(li + 1, "p", mi=mi_next)
                        agk = kprojA(li + 1, cs_t, mu16, rsS)
                        agv = vprojA(li + 1, cs_t, mu16, rstdT)
                        qproj(li + 1, cs_t, mu16, qmr,
                              128 if li + 1 == n_layers - 1 else 0)
                        load_K(agk)
                        load_V(agv)
                else:
                    with nc.named_scope("xag"):
                        x_in = dp.tile([128, 1024], BF16, tag="xin", name="xin")
                        nc.sync.dma_start(
                            x_in[:].rearrange("p (a k) -> p a k", a=8),
                            xb[:, :, 128:RPC])
                        agx = dp.tile([NC, 128, 1024], BF16, tag="xout",
                                      name="xout", addr_space="Shared")
                        nc.gpsimd.collective_compute(
                            "AllGather", mybir.AluOpType.bypass,
                            replica_groups=[list(range(NC))],
                            ins=[x_in.opt()], outs=[agx.opt()])

            # ---- target logits: local mask rows vs their target embeddings ----
            with nc.named_scope("tlogit"):
                Et = pp.tile([128, 8, 128], BF16, name="Et")
                nc.sync.dma_start(Et[:], etT.ap())
                tlp = psA.tile([1, 128], F32, name="tlp", tag="A")
                for a in range(8):
                    P = tp.tile([128, 128], F32, tag="P", name=f"P{a}")
                    nc.vector.tensor_tensor(P[:], xb[:, a, 128:RPC], Et[:, a, :],
                                            mybir.AluOpType.mult)
                    nc.tensor.matmul(tlp[:], ones[:], P[:],
                                     start=(a == 0), stop=(a == 7))
                tl_sb = sp.tile([1, 128], F32, tag="tlsb", name="tlsb", bufs=1)
                nc.vector.tensor_copy(tl_sb[:], tlp[:])
                nc.sync.dma_start(tlogit_o.ap(), tl_sb[:])

            # ---- unembedding (vocab-sharded) ----
            with nc.named_scope("unembed"):
                NV = 500
                xfull = pp.tile([128, 8, 8, 128], BF16, name="xfull")
                se_parts = pp.tile([128, 8, 8], F32, name="separts")
                for t in range(8):
                    nc.sync.dma_start(
                        xfull[:, :, t, :],
                        agx[t].rearrange("p (a k) -> p a k", a=8))
                for n in range(8):
                    ch = ep.tile([128, 8, NV], BF16, tag="emb", name=f"ec{n}")
                    nc.sync.dma_start(ch[:], embc.ap()[n])
                    for t in range(8):
                        ps = psA.tile([128, NV], F32, name=f"lg{n}{t}", tag="A")
                        for a in range(8):
                            nc.tensor.matmul(ps[:], xfull[:, a, t, :], ch[:, a, :],
                                             start=(a == 0), stop=(a == 7))
                        Esc = ep.tile([128, NV], BF16, tag="esc", name=f"esc{n}{t}")
                        nc.scalar.activation(Esc[:], ps[:],
                                             mybir.ActivationFunctionType.Exp,
                                             accum_out=se_parts[:, n, t : t + 1])
                se = sp.tile([128, 8], F32, tag="se", name="se")
                for t in range(8):
                    nc.vector.reduce_sum(se[:, t : t + 1], se_parts[:, :, t],
                                         axis=mybir.AxisListType.X)
                nc.sync.dma_start(sumexp_o.ap(), se[:])

    nc.finalize()
    return nc


def _rows_of(j):
    tok = np.arange(128 * j, 128 * j + 128)
    return np.concatenate([tok, 512 + tok])


def _qmask_rows(rows):
    """[2, 256] keep-masks (block, ar) for the given global row ids."""
    m = np.ones((2, len(rows)), np.float32)
    for i, g in enumerate(rows):
        if T - 2 * CSL <= g < T:
            m[0, i] = 0.0
        if g == T - 1 or (g >= T and (g - T) % CSL == CSL - 1):
            m[1, i] = 0.0
    return m


def _prep(inputs):
    """Host-side input prep -> per-core in_maps."""
    f = {k: np.asarray(v) for k, v in inputs.items()}
    tok_ids = f["tok_ids"].astype(np.int64)
    tok_emb = f["tok_emb"].astype(np.float32)
    pos_emb = f["pos_emb"].astype(np.float32)
    mask_tokens = f["mask_tokens"].astype(np.float32)

    # x0 [B, S, D]
    x0 = np.empty((B, S, D), np.float32)
    for b in range(B):
        x0[b, :T] = tok_emb[tok_ids[b]]
        x0[b, T:] = np.tile(mask_tokens[0], (T // CSL, 1))
    x0 += pos_emb[np.arange(S) % T][None]

    def stack(name):
        return np.concatenate([f["b_" + name], f["d_" + name]], axis=0)

    wqkv = stack("wqkv").astype(np.float32)
    wqkv_s = wqkv.copy()
    wqkv_s[:, :, :D] /= np.sqrt(DH)
    wo_s = stack("wo").astype(np.float32)
    w1_s = stack("w1").astype(np.float32)
    w2_s = stack("w2").astype(np.float32)

    # negated column sums for the rank-1 LN-mean correction
    csqkv_h = np.ascontiguousarray(
        (-wqkv_s.sum(axis=1))[:, None, :]).astype(BF)     # [6, 1, 3072]
    csw1_h = np.ascontiguousarray(
        (-w1_s.sum(axis=1))[:, None, :]).astype(BF)       # [6, 1, 4096]

    def chunk_cols(w, ncols):
        # [NL, D, M] -> [NL, M//ncols, 128, 8, ncols]  (row d = a*128+p)
        nl, d, m = w.shape
        out = w.reshape(nl, 8, 128, m // ncols, ncols).transpose(0, 3, 2, 1, 4)
        return np.ascontiguousarray(out).astype(BF)

    wqkvc = chunk_cols(wqkv_s, 512)                      # [6, 6, 128, 8, 512]
    woc = chunk_cols(wo_s, 512)                          # [6, 2, 128, 8, 512]
    w1cc = chunk_cols(w1_s, 512)                         # [6, 8, 128, 8, 512]
    # w2 (x32, fp8e4 DoubleRow pairs): [NL, DFF, D] ->
    # [6, 8 m2, 128 p, 16 kp, 2 j, 128 q], dff = 256*kp + 128*j + p
    w2r = (32.0 * w2_s).reshape(NLAYERS, 16, 2, 128, 8, 128)
    w2cc = np.ascontiguousarray(
        w2r.transpose(0, 4, 3, 1, 2, 5)).astype(F8)

    # layer-0 K/V per elem (host precompute; LN g=1 b=0)
    mu = x0.mean(-1, keepdims=True)
    var = x0.var(-1, keepdims=True)
    h0 = (x0 - mu) / np.sqrt(var + 1e-5)
    K0 = h0 @ wqkv[0, :, D : 2 * D]                      # [B, S, D] (unscaled)
    V0 = h0 @ wqkv[0, :, 2 * D : 3 * D]

    # per-core targets + their embeddings in local row order
    tgt = np.full(NC * 128, -1, np.int64)
    etTs = []
    for c in range(NC):
        e, j = c // 4, c % 4
        et_f = np.zeros((128, D), np.float32)
        for p in range(128):
            g = T + 128 * j + p
            if g >= T + 1:
                tid = tok_ids[e, g - T - 1]
                tgt[128 * c + p] = tid
                et_f[p] = tok_emb[tid]
        # local etT layout [128 dp, 8 a, 128 p]
        etTs.append(np.ascontiguousarray(
            et_f.T.reshape(8, 128, 128).transpose(1, 0, 2)).astype(BF))

    embT = tok_emb.T.astype(np.float32)                  # [D, V]

    in_maps = []
    for c in range(NC):
        e, j = c // 4, c % 4
        rows = _rows_of(j)
        x0T = np.ascontiguousarray(x0[e][rows].T)        # [D, 256] f32

        kv0c = np.empty((NG, 128, KVFREE), F8)
        for r in range(NG):
            rr = _rows_of(r)
            kT = K0[e][rr].T.reshape(8, 128, RPC).transpose(1, 0, 2)  # [p, a, k]
            kv0c[r, :, 0:KCH] = kT.reshape(128, KCH).astype(F8)
            vpart = np.empty((128, 2, H, DH + 1), np.float32)
            vloc = V0[e][rr].reshape(2, 128, H, DH).transpose(1, 0, 2, 3)
            vpart[:, :, :, 0:DH] = vloc
            vpart[:, :, :, DH] = 1.0
            kv0c[r, :, KCH:KVFREE] = vpart.reshape(128, VCH).astype(F8)

        # vocab shard chunks [n, p, a, v] = emb_sh[a*128+p, 500n+v]
        emb_sh = embT[:, VS * c : VS * (c + 1)]          # [D, 4000]
        embcc = np.ascontiguousarray(
            emb_sh.reshape(8, 128, 8, 500).transpose(2, 1, 0, 3)).astype(BF)

        in_maps.append({
            "x0": x0T,
            "kv0": kv0c,
            "wqkvc": wqkvc, "woc": woc, "w1c": w1cc, "w2c": w2cc,
            "csqkv": csqkv_h, "csw1": csw1_h,
            "qm": _qmask_rows(rows),
            "embc": embcc,
            "etT": etTs[c],
        })
    return in_maps, tgt


def _combine(results, tgt):
    se = np.zeros((NC * 128,), np.float64)
    for c in range(NC):
        # sumexp out [128 p, 8 t] -> flat tk = t*128 + p
        se += results[c]["sumexp"].astype(np.float64).T.reshape(-1)
    tl = np.concatenate(
        [results[c]["tlogit"].astype(np.float64).reshape(-1) for c in range(NC)])
    valid = tgt >= 0
    lse = np.log(se[valid])
    return np.float32(np.mean(lse - tl[valid]))


def kernel(**inputs):
    if "nc" not in _CACHE:
        _CACHE["nc"] = _build_nc()
    nc = _CACHE["nc"]
    in_maps, tgt = _prep(inputs)
    res = run_bass_kernel_spmd(nc, in_maps, core_ids=list(range(NC)))
    return _combine(res.results, tgt)
